# revision 21
# baseline (speedup 1.0000x reference)
"""BitSwarmLinear Trainium2 kernel.

Computation (reference):
    swarm_sum = population.sum(axis=2)          # (out, in)
    w_eff     = sign(swarm_sum), sign(0) -> +1  # (out, in), +-1
    y         = einsum("bsi,oi->bso", x, w_eff) # (4, 4096, out)

Distribution (8 NeuronCores, tensor-parallel on out_features):
    - population sharded on out_features: each core gets its 256 rows,
      reduces + binarizes them and computes its 256 output columns.
    - x replicated to every core, staged pre-transposed/tiled so the
      contraction dim lands on SBUF partitions with fully-contiguous DMA.
    - outputs gathered on the host along the feature dim.

Precision/speed split (the key trick): the PE's only >1x datatype path on
TRN2 is fp8e4/e5 + perf_mode=DoubleRow (2 weights/cell, 2 MACs/cycle).
Full e4m3 x quantization costs 2.66% rel err (> the 2e-2 gate), so the
contraction is split: in-features [0,1024) stay bf16 (8 k-tiles), and
in-features [1024,2048) are e4m3 (4 DoubleRow double-tiles, rhs free dim
1024 = the fp8 moving max). Measured rel err 1.89e-2; PE cycles per
512-token PSUM group drop 8288 -> 6464 (0.78x) and x HBM traffic drops
64 MB -> 48 MB per core (the baseline ran at ~91% DMA occupancy, so both
rooflines move together).

Host staging (lossless / layout-only for pop; x is cast to bf16/e4m3):
    - population bits 2-bit-packed: plane q's byte holds swarm bits
      {q, 8+q, 16+q, 24+q} at bit positions 0/2/4/6, laid out IN-major
      [plane, in%128, in//128, out] so the reduction output lands directly
      in matmul-rhs orientation (no PE transpose). 4.2 MB/core (4x less
      than u8 planes).
    - x -> bf16 x^T tiles [tb, 128 ki, 8 ko, TB tok] for the bf16 half and
      e4m3 x^T pair-tiles [tb, 128 ki, 4 d, 2 j, TB tok] for the fp8 half
      (j indexes the DoubleRow k-pair slot; contiguous lines either way).
    - y returns bf16 tile-major; host restores [b, s, out] f32.

Per-core device pipeline:
    1. Eight pop plane DMAs (525 KB each, alternating SP/ACT HWDGE rings,
       all issued upfront). DVE sums plane sets A={p0..p2}, B={p3..p5},
       C={p6,p7} as uint16 lanes (2-bit fields stay <= 3: carry-free,
       exact in the fp32 ALU), then unpacks 2->4 bit (nibbles <= 12) and
       4->8 bit (bytes <= 32 = the swarm popcount). The whole chain is
       split by ko-half with the fp8 half first, so w_f8 closes ~10 us
       before w_bf.
    2. Binarize in ONE op per W half: w = (count >= 16) - 0.5 = +-0.5
       (sign(0)->+1 preserved; +-0.5 exact in e4m3/bf16); the PSUM-drain
       cast multiplies y by 2 (exact power-of-2, zero precision cost).
    3. The first two token blocks' DoubleRow matmuls are emitted ahead of
       everything (start=True, PSUM groups left open): they start as soon
       as w_f8 + the sliver-gated x8 tiles land, fill the PE's head
       window while the bf16 W half finalizes, and warm HAM. Body: per
       (g, blk) PSUM group, 4 DR MMs then 8 bf16 MMs (PE queue in-order;
       fp8 operands are ready first). DVE drains PSUM (x2 scale, bf16);
       y stores alternate HWDGE rings.
"""

import os
import sys

import numpy as np

for _p in ("/root/.axon_site/_ro/trn_rl_repo", "/opt/trn_rl_repo"):
    if os.path.isdir(_p) and _p not in sys.path:
        sys.path.append(_p)

import ml_dtypes

# bass_utils' axon trace path imports antenv.axon_hooks, which this image
# lacks. Provide it (backed by the ctypes NTFF hook) so running with
# BASS_TRACE=1 works instead of crashing on the import.
try:
    import antenv.axon_hooks  # noqa: F401
except ImportError:
    try:
        import types as _types

        from trn_agent_boot.trn_boot import _ntff_profile_via_ctypes

        _hooks = _types.ModuleType("antenv.axon_hooks")
        _ntff_hook = _ntff_profile_via_ctypes("/opt/axon/libaxon_pjrt.so")
        _hooks.get_axon_ntff_profile_hook = lambda: _ntff_hook
        _hooks.set_axon_ntff_profile_hook = lambda h: None
        sys.modules["antenv.axon_hooks"] = _hooks
    except Exception:
        pass

import concourse.bass as bass  # noqa: F401  (AP helpers)
import concourse.mybir as mybir
import concourse.tile as tile
from concourse import bacc
from concourse.bass_utils import run_bass_kernel_spmd

P = 128
IN_F = 2048
SWARM = 32
OUT_F = 2048
N_CORES = 8
OUT_C = OUT_F // N_CORES  # 256 out features per core
TOKENS = 4 * 4096

F32 = mybir.dt.float32
BF16 = mybir.dt.bfloat16
F8E4 = mybir.dt.float8e4
U8 = mybir.dt.uint8
U16 = mybir.dt.uint16

# token-block per x tile / output store
TB = 1024
# x prefetch depth
XT_BUFS = 4
# 2-bit-packed pop planes, one per chunk DMA
PK_CHUNKS = 8
# contraction split: k-tiles 0..KO_BF-1 bf16, the rest e4m3 DoubleRow pairs
KO_BF = 8
D8 = (16 - KO_BF) // 2  # fp8 double-tiles (2 k-tiles each)
DR = mybir.MatmulPerfMode.DoubleRow


def build_nc(tokens: int = TOKENS, out_c: int = OUT_C, in_f: int = IN_F,
             reps: int = 1):
    """Build the per-core Bass program (same program on all 8 cores).

    reps>1 repeats the whole pipeline back-to-back (timing harness only)."""
    ko_tiles = in_f // P          # 16 K-tiles
    tb_count = tokens // TB

    nc = bacc.Bacc(
        "TRN2",
        target_bir_lowering=False,
        debug=False,
        enable_asserts=False,
        num_devices=N_CORES,
    )

    xT = nc.dram_tensor("xT", [tb_count, P, KO_BF, TB], BF16,
                        kind="ExternalInput")
    x8 = nc.dram_tensor("x8", [tb_count, P, D8, 2, TB], F8E4,
                        kind="ExternalInput")
    pop = nc.dram_tensor("pop", [PK_CHUNKS, P, ko_tiles, out_c],
                         U8, kind="ExternalInput")
    # y^T tile-major: [tb, out%128, out//128, tok-in-block]
    y = nc.dram_tensor("y", [tb_count, P, out_c // P, TB], BF16,
                       kind="ExternalOutput")

    xr = xT.ap()                                              # [tb,128,8,TB]
    x8r = x8.ap()                                             # [tb,128,4,2,TB]
    pr = pop.ap()                                             # [8,128,2,ko,oc]
    yr = y.ap()                                               # [tb,128,m,TB]

    with tile.TileContext(nc) as tc:
        with (
            tc.tile_pool(name="pops", bufs=PK_CHUNKS) as pop_pool,
            tc.tile_pool(name="red", bufs=1) as red_pool,
            tc.tile_pool(name="wsb", bufs=1) as w_pool,
            tc.tile_pool(name="xt", bufs=XT_BUFS) as x_pool,
            tc.tile_pool(name="x8t", bufs=XT_BUFS) as x8_pool,
            tc.tile_pool(name="ystage", bufs=3) as y_pool,
            tc.tile_pool(name="psum_y", bufs=8, space="PSUM") as psum_pool,
        ):
            for _rep in range(reps):
                _emit_body(
                    nc, pop_pool, red_pool, w_pool, x_pool, x8_pool, y_pool,
                    psum_pool, pr, xr, x8r, yr, ko_tiles, tb_count,
                    out_c, in_f,
                )

    nc.compile()  # bacc register allocation / DCE — required before codegen
    return nc


def _emit_body(nc, pop_pool, red_pool, w_pool, x_pool, x8_pool, y_pool,
               psum_pool, pr, xr, x8r, yr, ko_tiles, tb_count, out_c, in_f):
    # W tiles: bf16 [in(part), ko, out] + e4m3 pairs [in(part), d, j, out]
    w_bf = w_pool.tile([P, KO_BF, out_c], BF16, tag="wbf")
    w_f8 = w_pool.tile([P, D8, 2, out_c], F8E4, tag="wf8")

    # ---- Stage 1: swarm reduction over 2-bit-packed planes (all DVE).
    # Plane q's byte holds swarm bits {q, 8+q, 16+q, 24+q} at bit positions
    # 0/2/4/6. Sets A={p0,p1,p2}, B={p3,p4,p5}, C={p6,p7} are summed as
    # uint16 lane views (2-bit fields stay <= 3 per set -> no carries;
    # everything is exact in the fp32 ALU). The whole chain is split by
    # ko-half with the fp8 half (k-tiles 8..15) FIRST, so w_f8 closes
    # ~10 us before w_bf and the early DoubleRow matmuls fill that window.
    oc_l = out_c // 2  # uint16 lanes per ko row
    accA = red_pool.tile([P, ko_tiles, oc_l], U16, tag="accA")
    accB = red_pool.tile([P, ko_tiles, oc_l], U16, tag="accB")
    accC = red_pool.tile([P, ko_tiles, oc_l], U16, tag="accC")
    uA = red_pool.tile([P, ko_tiles, oc_l], U16, tag="uA")
    uB = red_pool.tile([P, ko_tiles, oc_l], U16, tag="uB")
    uC = red_pool.tile([P, ko_tiles, oc_l], U16, tag="uC")
    q1 = red_pool.tile([P, ko_tiles, oc_l], U16, tag="q1")
    q2 = red_pool.tile([P, ko_tiles, oc_l], U16, tag="q2")
    tS = red_pool.tile([P, ko_tiles, oc_l], U16, tag="tS")

    SHR = mybir.AluOpType.logical_shift_right
    AND = mybir.AluOpType.bitwise_and
    ISGE = mybir.AluOpType.is_ge
    MUL = mybir.AluOpType.mult
    SUB = mybir.AluOpType.subtract

    # All 8 chunk DMAs issue upfront (bufs=8 -> no buffer-reuse
    # throttling). The x-tile head loads are sliver-gated on the reduction
    # chain: an ungated DMA's transfer round-robins with the pop chunks at
    # packet granularity across DMA queues and steals pop bandwidth.
    pk_tiles = []
    for c in range(PK_CHUNKS):
        pt = pop_pool.tile([P, ko_tiles, out_c], U8, tag="pops")
        eng = nc.sync if c % 2 == 0 else nc.scalar
        eng.dma_start(pt[:], pr[c])
        pk_tiles.append(pt)
    pv = [t[:].bitcast(U16) for t in pk_tiles]

    xt_head = [x_pool.tile([P, KO_BF, TB], BF16, tag="xt", name=f"xth{i}")
               for i in range(XT_BUFS)]
    x8_head = [x8_pool.tile([P, D8, 2, TB], F8E4, tag="x8t", name=f"x8h{i}")
               for i in range(XT_BUFS)]

    def load_x8(x8t, tb, gate=None):
        if gate is not None:
            nc.gpsimd.tensor_copy(out=x8t[0:1, 0, 0, 0:32].bitcast(U16),
                                  in_=gate)
        eng = nc.sync if tb % 2 == 0 else nc.scalar
        eng.dma_start(x8t[:], x8r[tb])

    def load_xt(xt, tb, gate=None):
        half = KO_BF // 2
        for h in range(2):
            if gate is not None:
                nc.gpsimd.tensor_copy(out=xt[0:1, h * 4, 0:16],
                                      in_=gate.bitcast(BF16))
            eng = nc.sync if h == 0 else nc.scalar
            eng.dma_start(xt[:, h * half:(h + 1) * half, :],
                          xr[tb, :, h * half:(h + 1) * half, :])

    hl = oc_l // 2  # 64 lanes per out-column half
    cnt_u8 = q1[:].bitcast(U8)  # [128, ko, out_c] counts in [0, 32]

    def unpack2(dst, src, ks):
        nc.vector.tensor_scalar(out=tS[:, ks], in0=src[:, ks], scalar1=2,
                                scalar2=0x3333, op0=SHR, op1=AND)
        nc.vector.tensor_scalar(out=dst[:, ks], in0=src[:, ks],
                                scalar1=0x3333, scalar2=None, op0=AND)
        nc.vector.tensor_add(dst[:, ks], dst[:, ks], tS[:, ks])

    def half_tail(ks, g):
        """abq (<=24) + cq (<=8) + cnt (<=32) for one (ko-half, out-half)."""
        ls = slice(g * hl, (g + 1) * hl)
        nc.vector.tensor_scalar(out=tS[:, ks, ls], in0=uB[:, ks, ls],
                                scalar1=4, scalar2=0x0F0F, op0=SHR, op1=AND)
        nc.vector.tensor_scalar(out=q1[:, ks, ls], in0=uB[:, ks, ls],
                                scalar1=0x0F0F, scalar2=None, op0=AND)
        nc.vector.tensor_add(q1[:, ks, ls], q1[:, ks, ls], tS[:, ks, ls])
        nc.vector.tensor_scalar(out=tS[:, ks, ls], in0=uC[:, ks, ls],
                                scalar1=4, scalar2=0x0F0F, op0=SHR, op1=AND)
        nc.vector.tensor_scalar(out=q2[:, ks, ls], in0=uC[:, ks, ls],
                                scalar1=0x0F0F, scalar2=None, op0=AND)
        nc.vector.tensor_add(q2[:, ks, ls], q2[:, ks, ls], tS[:, ks, ls])
        nc.vector.tensor_add(q1[:, ks, ls], q1[:, ks, ls], q2[:, ks, ls])

    def binarize(dst, src):
        # count >= 16 <=> swarm_sum >= 0. W is stored as +-0.5 (exact in
        # e4m3/bf16) in ONE op; the PSUM-drain cast multiplies y by 2
        # (exact power-of-2 scaling, no precision change).
        nc.vector.tensor_scalar(out=dst, in0=src, scalar1=16, scalar2=0.5,
                                op0=ISGE, op1=SUB)

    ksF = slice(KO_BF, ko_tiles)   # fp8 half: k-tiles 8..15
    ksB = slice(0, KO_BF)          # bf16 half
    # fp8-half chain first, with the bf16-half set adds slotted into the
    # chunk-arrival stalls of the fp8 chain; the unpack/tail for the bf16
    # half runs after the fp8 W has closed (the early DR matmuls cover it).
    nc.vector.tensor_add(accA[:, ksF], pv[0][:, ksF], pv[1][:, ksF])
    nc.vector.tensor_add(accA[:, ksB], pv[0][:, ksB], pv[1][:, ksB])
    nc.vector.tensor_add(accA[:, ksF], accA[:, ksF], pv[2][:, ksF])
    nc.vector.tensor_add(accA[:, ksB], accA[:, ksB], pv[2][:, ksB])
    unpack2(uA, accA, ksF)                              # A1 (F)
    nc.vector.tensor_add(accB[:, ksF], pv[3][:, ksF], pv[4][:, ksF])
    nc.vector.tensor_add(accB[:, ksF], accB[:, ksF], pv[5][:, ksF])
    nc.vector.tensor_add(accB[:, ksB], pv[3][:, ksB], pv[4][:, ksB])
    nc.vector.tensor_add(accB[:, ksB], accB[:, ksB], pv[5][:, ksB])
    nc.vector.tensor_add(accC[:, ksF], pv[6][:, ksF], pv[7][:, ksF])
    nc.vector.tensor_add(accC[:, ksB], pv[6][:, ksB], pv[7][:, ksB])
    # pop stream is fully consumed here: release the first x tiles
    gateF = accC[0:1, KO_BF, 0:16]
    load_x8(x8_head[0], 0, gateF)
    load_x8(x8_head[1], 1, gateF)
    load_xt(xt_head[0], 0, gateF)
    load_xt(xt_head[1], 1, gateF)
    unpack2(uB, accB, ksF)                              # B1 (F)
    nc.vector.tensor_add(uB[:, ksF], uB[:, ksF], uA[:, ksF])   # ab <= 12
    unpack2(uC, accC, ksF)                              # C1 (F) <= 4
    gateF2 = uC[0:1, KO_BF, 0:16]
    for i in range(2, XT_BUFS):
        load_x8(x8_head[i], i, gateF2)
        load_xt(xt_head[i], i, gateF2)
    for g in range(2):
        gs = slice(g * P, (g + 1) * P)
        half_tail(ksF, g)
        binarize(w_f8[:, :, :, gs], cnt_u8[:, KO_BF:ko_tiles, gs])
    # bf16-half unpack + tails (under cover of the early DR matmuls)
    unpack2(uA, accA, ksB)
    unpack2(uB, accB, ksB)
    nc.vector.tensor_add(uB[:, ksB], uB[:, ksB], uA[:, ksB])
    unpack2(uC, accC, ksB)
    for g in range(2):
        gs = slice(g * P, (g + 1) * P)
        half_tail(ksB, g)
        binarize(w_bf[:, :, gs], cnt_u8[:, 0:KO_BF, gs])

    # ---- Stage 2.5: early DoubleRow matmuls. w_f8 + the x8 tiles land
    # while the bf16 W half + xt0 are still in flight, so the first
    # ED_TBS token blocks' DR matmuls are emitted ahead of everything:
    # they open the PSUM accumulation groups (start=True, no stop), fill
    # the PE's otherwise-idle head window, and warm HAM before the body.
    oc_g = out_c // P
    n_blk = TB // 512
    ED_TBS = 2  # 2 tb x 4 groups = all 8 PSUM banks held open
    ps_early = {}
    # g-major: the g0 DR matmuls run while w_f8's g1 half still finalizes
    for g in range(oc_g):
        for tb in range(ED_TBS):
            for blk in range(n_blk):
                ps = psum_pool.tile([P, 512], F32, tag="yps")
                ps_early[(tb, g, blk)] = ps
                for dd in range(D8):
                    nc.tensor.matmul(
                        ps[:],
                        w_f8[:, dd, :, g * P:(g + 1) * P],
                        x8_head[tb][:, dd, :, blk * 512:(blk + 1) * 512],
                        start=(dd == 0),
                        stop=False,
                        perf_mode=DR,
                    )

    # ---- Stage 3: stream x tiles, matmul, store y (bf16)
    for tb in range(tb_count):
        if tb < XT_BUFS:
            xt = xt_head[tb]   # loads already issued in the head
            x8t = x8_head[tb]
        else:
            xt = x_pool.tile([P, KO_BF, TB], BF16, tag="xt")
            x8t = x8_pool.tile([P, D8, 2, TB], F8E4, tag="x8t")
            load_x8(x8t, tb)
            load_xt(xt, tb)
        # W-stationary matmuls: 4 e4m3 DoubleRow double-tiles + 8 bf16
        # k-tiles accumulate one PSUM bank [128 out, 512 tok]. DR first:
        # x8/w_f8 are ready before xt/w_bf, and the PE queue is in-order.
        ystage = y_pool.tile([P, oc_g, TB], BF16, tag="ys")
        for g in range(oc_g):
            for blk in range(n_blk):
                if tb < ED_TBS:
                    ps = ps_early[(tb, g, blk)]  # DR part already emitted
                else:
                    ps = psum_pool.tile([P, 512], F32, tag="yps")
                    for dd in range(D8):
                        nc.tensor.matmul(
                            ps[:],
                            w_f8[:, dd, :, g * P:(g + 1) * P],
                            x8t[:, dd, :, blk * 512:(blk + 1) * 512],
                            start=(dd == 0),
                            stop=False,
                            perf_mode=DR,
                        )
                for k in range(KO_BF):
                    nc.tensor.matmul(
                        ps[:],
                        w_bf[:, k, g * P:(g + 1) * P],
                        xt[:, k, blk * 512:(blk + 1) * 512],
                        start=False,
                        stop=(k == KO_BF - 1),
                    )
                nc.vector.tensor_scalar(
                    out=ystage[:, g, blk * 512:(blk + 1) * 512], in0=ps[:],
                    scalar1=2.0, scalar2=None, op0=mybir.AluOpType.mult,
                )
            eng = nc.sync if (tb + g) % 2 == 0 else nc.scalar
            eng.dma_start(yr[tb, :, g:g + 1, :], ystage[:, g:g + 1, :])


_NC_CACHE: dict = {}


def _get_nc(tokens=TOKENS, out_c=OUT_C, in_f=IN_F):
    key = (tokens, out_c, in_f)
    if key not in _NC_CACHE:
        _NC_CACHE[key] = build_nc(*key)
    return _NC_CACHE[key]


def stage_x(x: np.ndarray, tokens: int, in_f: int):
    """x [b, s, in] f32 -> (bf16 tiles [tb, 128 ki, 8 ko, TB] for
    in-features [0, 1024), e4m3 pair-tiles [tb, 128 ki, 4 d, 2 j, TB] for
    in-features [1024, 2048))."""
    xf = np.ascontiguousarray(x.reshape(tokens, in_f).T)  # [in, tokens]
    tb = tokens // TB
    kb = KO_BF * P
    xbf = xf[:kb].astype(ml_dtypes.bfloat16)
    # (ko ki) (tb t) -> tb ki ko t
    x_bf = np.ascontiguousarray(
        xbf.reshape(KO_BF, P, tb, TB).transpose(2, 1, 0, 3)
    )
    x8f = xf[kb:].astype(ml_dtypes.float8_e4m3fn)
    # (d j ki) (tb t) -> tb ki d j t
    x_f8 = np.ascontiguousarray(
        x8f.reshape(D8, 2, P, tb, TB).transpose(3, 2, 0, 1, 4)
    )
    return x_bf, x_f8


def stage_pop_slice(pop_c: np.ndarray):
    """pop slice [out_c, in, 32] (+-1.0 f32) -> 2-bit-packed swarm planes
    [8 chunk, 128 p, ko, out_c] u8. Lossless bit-repack: plane q's byte
    holds swarm bits {q, 8+q, 16+q, 24+q} at bit positions 0/2/4/6, so
    summing <=3 planes keeps every 2-bit field carry-free."""
    out_c, in_f, _ = pop_c.shape
    ko = in_f // P
    bits = (pop_c > 0).astype(np.uint8).transpose(2, 1, 0)  # [32, in, out_c]
    planes = (bits[0:8] | (bits[8:16] << 2) | (bits[16:24] << 4)
              | (bits[24:32] << 6))  # [8 q, in, out_c]
    # [8 q, (ko p) in, oc] -> [8 c, 128 p, ko, oc]
    arr = planes.reshape(8, ko, P, out_c).transpose(0, 2, 1, 3)
    return np.ascontiguousarray(arr)


def unstage_y(y_dev: np.ndarray, tokens: int, out_c: int):
    """y^T [tb, 128 o, g, TB t] bf16 -> [tokens, out_c] f32
    (token = tb*TB + t, out = g*128 + o)."""
    return (
        y_dev.astype(np.float32)
        .transpose(0, 3, 2, 1)
        .reshape(tokens, out_c)
    )


def prep_inputs(x: np.ndarray, population: np.ndarray):
    tokens = x.shape[0] * x.shape[1]
    in_f = x.shape[2]
    x_bf, x_f8 = stage_x(x, tokens, in_f)
    out_c = population.shape[0] // N_CORES
    in_maps = []
    for c in range(N_CORES):
        pop_c = stage_pop_slice(population[c * out_c:(c + 1) * out_c])
        in_maps.append({"xT": x_bf, "x8": x_f8, "pop": pop_c})
    return in_maps, tokens, out_c, in_f


def kernel(x: np.ndarray, population: np.ndarray):
    in_maps, tokens, out_c, in_f = prep_inputs(x, population)
    nc = _get_nc(tokens, out_c, in_f)
    res = run_bass_kernel_spmd(nc, in_maps, core_ids=list(range(N_CORES)))
    y_full = np.concatenate(
        [unstage_y(r["y"], tokens, out_c) for r in res.results], axis=1
    )
    return y_full.reshape(x.shape[0], x.shape[1], population.shape[0])


# revision 23
# speedup vs baseline: 1.0295x; 1.0295x over previous
"""BitSwarmLinear Trainium2 kernel.

Computation (reference):
    swarm_sum = population.sum(axis=2)          # (out, in)
    w_eff     = sign(swarm_sum), sign(0) -> +1  # (out, in), +-1
    y         = einsum("bsi,oi->bso", x, w_eff) # (4, 4096, out)

Distribution (8 NeuronCores, tensor-parallel on out_features):
    - population sharded on out_features: each core gets its 256 rows,
      reduces + binarizes them and computes its 256 output columns.
    - x replicated to every core, staged pre-transposed/tiled so the
      contraction dim lands on SBUF partitions with fully-contiguous DMA.
    - outputs gathered on the host along the feature dim.

Precision/speed split (the key trick): the PE's only >1x datatype path on
TRN2 is fp8e4/e5 + perf_mode=DoubleRow (2 weights/cell, 2 MACs/cycle).
Full e4m3 x quantization costs 2.66% rel err (> the 2e-2 gate), so the
contraction is split: in-features [0,1024) stay bf16 (8 k-tiles), and
in-features [1024,2048) are e4m3 (4 DoubleRow double-tiles, rhs free dim
1024 = the fp8 moving max). Measured rel err 1.89e-2; PE cycles per
512-token PSUM group drop 8288 -> 6464 (0.78x) and x HBM traffic drops
64 MB -> 48 MB per core (the baseline ran at ~91% DMA occupancy, so both
rooflines move together).

Host staging (lossless / layout-only for pop; x is cast to bf16/e4m3):
    - population bits 2-bit-packed: plane q's byte holds swarm bits
      {q, 8+q, 16+q, 24+q} at bit positions 0/2/4/6, laid out IN-major
      [plane, in%128, in//128, out] so the reduction output lands directly
      in matmul-rhs orientation (no PE transpose). 4.2 MB/core (4x less
      than u8 planes).
    - x -> bf16 x^T tiles [tb, 128 ki, 8 ko, TB tok] for the bf16 half and
      e4m3 x^T pair-tiles [tb, 128 ki, 4 d, 2 j, TB tok] for the fp8 half
      (j indexes the DoubleRow k-pair slot; contiguous lines either way).
    - y returns bf16 tile-major; host restores [b, s, out] f32.

Per-core device pipeline:
    1. Eight pop plane DMAs (525 KB each, alternating SP/ACT HWDGE rings,
       all issued upfront). DVE sums plane sets A={p0..p2}, B={p3..p5},
       C={p6,p7} as uint16 lanes (2-bit fields stay <= 3: carry-free,
       exact in the fp32 ALU), then unpacks 2->4 bit (nibbles <= 12) and
       4->8 bit (bytes <= 32 = the swarm popcount). The whole chain is
       split by ko-half with the fp8 half first, so w_f8 closes ~10 us
       before w_bf.
    2. Binarize in ONE op per W half: w = (count >= 16) - 0.5 = +-0.5
       (sign(0)->+1 preserved; +-0.5 exact in e4m3/bf16); the PSUM-drain
       cast multiplies y by 2 (exact power-of-2, zero precision cost).
    3. The first two token blocks' DoubleRow matmuls are emitted ahead of
       everything (start=True, PSUM groups left open): they start as soon
       as w_f8 + the sliver-gated x8 tiles land, fill the PE's head
       window while the bf16 W half finalizes, and warm HAM. Body: per
       (g, blk) PSUM group, 4 DR MMs then 8 bf16 MMs (PE queue in-order;
       fp8 operands are ready first). DVE drains PSUM (x2 scale, bf16);
       y stores alternate HWDGE rings.
"""

import os
import sys

import numpy as np

for _p in ("/root/.axon_site/_ro/trn_rl_repo", "/opt/trn_rl_repo"):
    if os.path.isdir(_p) and _p not in sys.path:
        sys.path.append(_p)

import ml_dtypes

# bass_utils' axon trace path imports antenv.axon_hooks, which this image
# lacks. Provide it (backed by the ctypes NTFF hook) so running with
# BASS_TRACE=1 works instead of crashing on the import.
try:
    import antenv.axon_hooks  # noqa: F401
except ImportError:
    try:
        import types as _types

        from trn_agent_boot.trn_boot import _ntff_profile_via_ctypes

        _hooks = _types.ModuleType("antenv.axon_hooks")
        _ntff_hook = _ntff_profile_via_ctypes("/opt/axon/libaxon_pjrt.so")
        _hooks.get_axon_ntff_profile_hook = lambda: _ntff_hook
        _hooks.set_axon_ntff_profile_hook = lambda h: None
        sys.modules["antenv.axon_hooks"] = _hooks
    except Exception:
        pass

import concourse.bass as bass  # noqa: F401  (AP helpers)
import concourse.mybir as mybir
import concourse.tile as tile
from concourse import bacc
from concourse.bass_utils import run_bass_kernel_spmd

P = 128
IN_F = 2048
SWARM = 32
OUT_F = 2048
N_CORES = 8
OUT_C = OUT_F // N_CORES  # 256 out features per core
TOKENS = 4 * 4096

F32 = mybir.dt.float32
BF16 = mybir.dt.bfloat16
F8E4 = mybir.dt.float8e4
U8 = mybir.dt.uint8
U16 = mybir.dt.uint16

# token-block per x tile / output store
TB = 1024
# x prefetch depth
XT_BUFS = 4
# 2-bit-packed pop planes, one per chunk DMA
PK_CHUNKS = 8
# contraction split: k-tiles 0..KO_BF-1 bf16, the rest e4m3 DoubleRow pairs
KO_BF = 8
D8 = (16 - KO_BF) // 2  # fp8 double-tiles (2 k-tiles each)
DR = mybir.MatmulPerfMode.DoubleRow


def build_nc(tokens: int = TOKENS, out_c: int = OUT_C, in_f: int = IN_F,
             reps: int = 1):
    """Build the per-core Bass program (same program on all 8 cores).

    reps>1 repeats the whole pipeline back-to-back (timing harness only)."""
    ko_tiles = in_f // P          # 16 K-tiles
    tb_count = tokens // TB

    nc = bacc.Bacc(
        "TRN2",
        target_bir_lowering=False,
        debug=False,
        enable_asserts=False,
        num_devices=N_CORES,
    )

    xT = nc.dram_tensor("xT", [tb_count, P, KO_BF, TB], BF16,
                        kind="ExternalInput")
    x8 = nc.dram_tensor("x8", [tb_count, P, D8, 2, TB], F8E4,
                        kind="ExternalInput")
    pop = nc.dram_tensor("pop", [PK_CHUNKS, P, ko_tiles, out_c],
                         U8, kind="ExternalInput")
    # y^T tile-major: [tb, out%128, out//128, tok-in-block]
    y = nc.dram_tensor("y", [tb_count, P, out_c // P, TB], BF16,
                       kind="ExternalOutput")

    xr = xT.ap()                                              # [tb,128,8,TB]
    x8r = x8.ap()                                             # [tb,128,4,2,TB]
    pr = pop.ap()                                             # [8,128,2,ko,oc]
    yr = y.ap()                                               # [tb,128,m,TB]

    with tile.TileContext(nc) as tc:
        with (
            tc.tile_pool(name="pops", bufs=PK_CHUNKS) as pop_pool,
            tc.tile_pool(name="red", bufs=1) as red_pool,
            tc.tile_pool(name="wsb", bufs=1) as w_pool,
            tc.tile_pool(name="xt", bufs=XT_BUFS) as x_pool,
            tc.tile_pool(name="x8t", bufs=XT_BUFS) as x8_pool,
            tc.tile_pool(name="ystage", bufs=3) as y_pool,
            tc.tile_pool(name="psum_y", bufs=8, space="PSUM") as psum_pool,
        ):
            for _rep in range(reps):
                _emit_body(
                    nc, pop_pool, red_pool, w_pool, x_pool, x8_pool, y_pool,
                    psum_pool, pr, xr, x8r, yr, ko_tiles, tb_count,
                    out_c, in_f,
                )

    nc.compile()  # bacc register allocation / DCE — required before codegen
    return nc


def _emit_body(nc, pop_pool, red_pool, w_pool, x_pool, x8_pool, y_pool,
               psum_pool, pr, xr, x8r, yr, ko_tiles, tb_count, out_c, in_f):
    # W tiles: bf16 [in(part), ko, out] + e4m3 pairs [in(part), d, j, out]
    w_bf = w_pool.tile([P, KO_BF, out_c], BF16, tag="wbf")
    w_f8 = w_pool.tile([P, D8, 2, out_c], F8E4, tag="wf8")

    # ---- Stage 1: swarm reduction over 2-bit-packed planes (all DVE).
    # Plane q's byte holds swarm bits {q, 8+q, 16+q, 24+q} at bit positions
    # 0/2/4/6. Sets A={p0,p1,p2}, B={p3,p4,p5}, C={p6,p7} are summed as
    # uint16 lane views (2-bit fields stay <= 3 per set -> no carries;
    # everything is exact in the fp32 ALU). The whole chain is split by
    # ko-half with the fp8 half (k-tiles 8..15) FIRST, so w_f8 closes
    # ~10 us before w_bf and the early DoubleRow matmuls fill that window.
    oc_l = out_c // 2  # uint16 lanes per ko row
    accA = red_pool.tile([P, ko_tiles, oc_l], U16, tag="accA")
    accB = red_pool.tile([P, ko_tiles, oc_l], U16, tag="accB")
    accC = red_pool.tile([P, ko_tiles, oc_l], U16, tag="accC")
    uA = red_pool.tile([P, ko_tiles, oc_l], U16, tag="uA")
    uB = red_pool.tile([P, ko_tiles, oc_l], U16, tag="uB")
    uC = red_pool.tile([P, ko_tiles, oc_l], U16, tag="uC")
    q1 = red_pool.tile([P, ko_tiles, oc_l], U16, tag="q1")
    q2 = red_pool.tile([P, ko_tiles, oc_l], U16, tag="q2")
    tS = red_pool.tile([P, ko_tiles, oc_l], U16, tag="tS")

    SHR = mybir.AluOpType.logical_shift_right
    AND = mybir.AluOpType.bitwise_and
    ISGE = mybir.AluOpType.is_ge
    MUL = mybir.AluOpType.mult
    SUB = mybir.AluOpType.subtract

    # All 8 chunk DMAs issue upfront (bufs=8 -> no buffer-reuse
    # throttling). The x-tile head loads are sliver-gated on the reduction
    # chain: an ungated DMA's transfer round-robins with the pop chunks at
    # packet granularity across DMA queues and steals pop bandwidth.
    pk_tiles = []
    for c in range(PK_CHUNKS):
        pt = pop_pool.tile([P, ko_tiles, out_c], U8, tag="pops")
        eng = nc.sync if c % 2 == 0 else nc.scalar
        eng.dma_start(pt[:], pr[c])
        pk_tiles.append(pt)
    pv = [t[:].bitcast(U16) for t in pk_tiles]

    xt_head = [x_pool.tile([P, KO_BF, TB], BF16, tag="xt", name=f"xth{i}")
               for i in range(XT_BUFS)]
    x8_head = [x8_pool.tile([P, D8, 2, TB], F8E4, tag="x8t", name=f"x8h{i}")
               for i in range(XT_BUFS)]

    def load_x8(x8t, tb, gate=None):
        if gate is not None:
            nc.gpsimd.tensor_copy(out=x8t[0:1, 0, 0, 0:32].bitcast(U16),
                                  in_=gate)
        eng = nc.sync if tb % 2 == 0 else nc.scalar
        eng.dma_start(x8t[:], x8r[tb])

    def load_xt(xt, tb, gate=None):
        half = KO_BF // 2
        for h in range(2):
            if gate is not None:
                nc.gpsimd.tensor_copy(out=xt[0:1, h * 4, 0:16],
                                      in_=gate.bitcast(BF16))
            eng = nc.sync if h == 0 else nc.scalar
            eng.dma_start(xt[:, h * half:(h + 1) * half, :],
                          xr[tb, :, h * half:(h + 1) * half, :])

    hl = oc_l // 2  # 64 lanes per out-column half
    cnt_u8 = q1[:].bitcast(U8)  # [128, ko, out_c] counts in [0, 32]

    def unpack2(dst, src, ks):
        nc.vector.tensor_scalar(out=tS[:, ks], in0=src[:, ks], scalar1=2,
                                scalar2=0x3333, op0=SHR, op1=AND)
        nc.vector.tensor_scalar(out=dst[:, ks], in0=src[:, ks],
                                scalar1=0x3333, scalar2=None, op0=AND)
        nc.vector.tensor_add(dst[:, ks], dst[:, ks], tS[:, ks])

    def half_tail(ks, g):
        """abq (<=24) + cq (<=8) + cnt (<=32) for one (ko-half, out-half)."""
        ls = slice(g * hl, (g + 1) * hl)
        nc.vector.tensor_scalar(out=tS[:, ks, ls], in0=uB[:, ks, ls],
                                scalar1=4, scalar2=0x0F0F, op0=SHR, op1=AND)
        nc.vector.tensor_scalar(out=q1[:, ks, ls], in0=uB[:, ks, ls],
                                scalar1=0x0F0F, scalar2=None, op0=AND)
        nc.vector.tensor_add(q1[:, ks, ls], q1[:, ks, ls], tS[:, ks, ls])
        nc.vector.tensor_scalar(out=tS[:, ks, ls], in0=uC[:, ks, ls],
                                scalar1=4, scalar2=0x0F0F, op0=SHR, op1=AND)
        nc.vector.tensor_scalar(out=q2[:, ks, ls], in0=uC[:, ks, ls],
                                scalar1=0x0F0F, scalar2=None, op0=AND)
        nc.vector.tensor_add(q2[:, ks, ls], q2[:, ks, ls], tS[:, ks, ls])
        nc.vector.tensor_add(q1[:, ks, ls], q1[:, ks, ls], q2[:, ks, ls])

    def binarize(dst, src):
        # count >= 16 <=> swarm_sum >= 0. W is stored as +-0.5 (exact in
        # e4m3/bf16) in ONE op; the PSUM-drain cast multiplies y by 2
        # (exact power-of-2 scaling, no precision change).
        nc.vector.tensor_scalar(out=dst, in0=src, scalar1=16, scalar2=0.5,
                                op0=ISGE, op1=SUB)

    ksF = slice(KO_BF, ko_tiles)   # fp8 half: k-tiles 8..15
    ksB = slice(0, KO_BF)          # bf16 half
    # fp8-half chain first, with the bf16-half set adds slotted into the
    # chunk-arrival stalls of the fp8 chain; the unpack/tail for the bf16
    # half runs after the fp8 W has closed (the early DR matmuls cover it).
    nc.vector.tensor_add(accA[:, ksF], pv[0][:, ksF], pv[1][:, ksF])
    nc.vector.tensor_add(accA[:, ksB], pv[0][:, ksB], pv[1][:, ksB])
    nc.vector.tensor_add(accA[:, ksF], accA[:, ksF], pv[2][:, ksF])
    nc.vector.tensor_add(accA[:, ksB], accA[:, ksB], pv[2][:, ksB])
    unpack2(uA, accA, ksF)                              # A1 (F)
    nc.vector.tensor_add(accB[:, ksF], pv[3][:, ksF], pv[4][:, ksF])
    nc.vector.tensor_add(accB[:, ksF], accB[:, ksF], pv[5][:, ksF])
    nc.vector.tensor_add(accC[:, ksF], pv[6][:, ksF], pv[7][:, ksF])
    # fp8-half pop is fully consumed here: release the first x tiles
    gateF = accC[0:1, KO_BF, 0:16]
    load_x8(x8_head[0], 0, gateF)
    load_x8(x8_head[1], 1, gateF)
    load_xt(xt_head[0], 0, gateF)
    load_xt(xt_head[1], 1, gateF)
    unpack2(uB, accB, ksF)                              # B1 (F)
    nc.vector.tensor_add(uB[:, ksF], uB[:, ksF], uA[:, ksF])   # ab <= 12
    unpack2(uC, accC, ksF)                              # C1 (F) <= 4
    gateF2 = uC[0:1, KO_BF, 0:16]
    for i in range(2, XT_BUFS):
        load_x8(x8_head[i], i, gateF2)
        load_xt(xt_head[i], i, gateF2)
    for g in range(2):
        gs = slice(g * P, (g + 1) * P)
        half_tail(ksF, g)
        binarize(w_f8[:, :, :, gs], cnt_u8[:, KO_BF:ko_tiles, gs])
    # bf16-half set adds + unpack + tails (under early-DR matmul cover)
    nc.vector.tensor_add(accB[:, ksB], pv[3][:, ksB], pv[4][:, ksB])
    nc.vector.tensor_add(accB[:, ksB], accB[:, ksB], pv[5][:, ksB])
    nc.vector.tensor_add(accC[:, ksB], pv[6][:, ksB], pv[7][:, ksB])
    unpack2(uA, accA, ksB)
    unpack2(uB, accB, ksB)
    nc.vector.tensor_add(uB[:, ksB], uB[:, ksB], uA[:, ksB])
    unpack2(uC, accC, ksB)
    for g in range(2):
        gs = slice(g * P, (g + 1) * P)
        half_tail(ksB, g)
        binarize(w_bf[:, :, gs], cnt_u8[:, 0:KO_BF, gs])

    # ---- HAM warm-up: garbage matmuls on dead pop bytes keep the PE
    # busy from when the fp8-half B-set closes (~18 us) until the early
    # DR matmuls start, so HAM is at K=8/8 for the entire real body.
    # PSUM garbage is never read: the real group reusing the bank opens
    # with start=True. lhsT/rhs read pk0 / accB's fp8-half (both dead or
    # read-only afterwards), so nothing in the pipeline is delayed.
    wu_lhsT = pk_tiles[0][:, 0, 0:256].bitcast(BF16)
    wu_rhs = accB[:, KO_BF:KO_BF + 4, :].bitcast(BF16)
    # shares the first early-DR group's PSUM bank: that group's start=True
    # erases the garbage (has_written cleared), so no 9th bank is needed
    ps_d = psum_pool.tile([P, 512], F32, tag="yps")
    for _ in range(30):
        nc.tensor.matmul(ps_d[:], wu_lhsT, wu_rhs, start=True, stop=True)

    # ---- Stage 2.5: early DoubleRow matmuls. w_f8 + the x8 tiles land
    # while the bf16 W half + xt0 are still in flight, so the first
    # ED_TBS token blocks' DR matmuls are emitted ahead of everything:
    # they open the PSUM accumulation groups (start=True, no stop), fill
    # the PE's otherwise-idle head window, and warm HAM before the body.
    oc_g = out_c // P
    n_blk = TB // 512
    ED_TBS = 2  # 2 tb x 4 groups = all 8 PSUM banks held open
    ps_early = {}
    # g-major: the g0 DR matmuls run while w_f8's g1 half still finalizes
    for g in range(oc_g):
        for tb in range(ED_TBS):
            for blk in range(n_blk):
                if (g, tb, blk) == (0, 0, 0):
                    ps = ps_d  # reuse the warm-up bank
                else:
                    ps = psum_pool.tile([P, 512], F32, tag="yps")
                ps_early[(tb, g, blk)] = ps
                for dd in range(D8):
                    nc.tensor.matmul(
                        ps[:],
                        w_f8[:, dd, :, g * P:(g + 1) * P],
                        x8_head[tb][:, dd, :, blk * 512:(blk + 1) * 512],
                        start=(dd == 0),
                        stop=False,
                        perf_mode=DR,
                    )

    # ---- Stage 3: stream x tiles, matmul, store y (bf16)
    for tb in range(tb_count):
        if tb < XT_BUFS:
            xt = xt_head[tb]   # loads already issued in the head
            x8t = x8_head[tb]
        else:
            xt = x_pool.tile([P, KO_BF, TB], BF16, tag="xt")
            x8t = x8_pool.tile([P, D8, 2, TB], F8E4, tag="x8t")
            load_x8(x8t, tb)
            load_xt(xt, tb)
        # W-stationary matmuls: 4 e4m3 DoubleRow double-tiles + 8 bf16
        # k-tiles accumulate one PSUM bank [128 out, 512 tok]. DR first:
        # x8/w_f8 are ready before xt/w_bf, and the PE queue is in-order.
        ystage = y_pool.tile([P, oc_g, TB], BF16, tag="ys")
        for g in range(oc_g):
            for blk in range(n_blk):
                if tb < ED_TBS:
                    ps = ps_early[(tb, g, blk)]  # DR part already emitted
                else:
                    ps = psum_pool.tile([P, 512], F32, tag="yps")
                    for dd in range(D8):
                        nc.tensor.matmul(
                            ps[:],
                            w_f8[:, dd, :, g * P:(g + 1) * P],
                            x8t[:, dd, :, blk * 512:(blk + 1) * 512],
                            start=(dd == 0),
                            stop=False,
                            perf_mode=DR,
                        )
                for k in range(KO_BF):
                    nc.tensor.matmul(
                        ps[:],
                        w_bf[:, k, g * P:(g + 1) * P],
                        xt[:, k, blk * 512:(blk + 1) * 512],
                        start=False,
                        stop=(k == KO_BF - 1),
                    )
                nc.vector.tensor_scalar(
                    out=ystage[:, g, blk * 512:(blk + 1) * 512], in0=ps[:],
                    scalar1=2.0, scalar2=None, op0=mybir.AluOpType.mult,
                )
            eng = nc.sync if (tb + g) % 2 == 0 else nc.scalar
            eng.dma_start(yr[tb, :, g:g + 1, :], ystage[:, g:g + 1, :])


_NC_CACHE: dict = {}


def _get_nc(tokens=TOKENS, out_c=OUT_C, in_f=IN_F):
    key = (tokens, out_c, in_f)
    if key not in _NC_CACHE:
        _NC_CACHE[key] = build_nc(*key)
    return _NC_CACHE[key]


def stage_x(x: np.ndarray, tokens: int, in_f: int):
    """x [b, s, in] f32 -> (bf16 tiles [tb, 128 ki, 8 ko, TB] for
    in-features [0, 1024), e4m3 pair-tiles [tb, 128 ki, 4 d, 2 j, TB] for
    in-features [1024, 2048))."""
    xf = np.ascontiguousarray(x.reshape(tokens, in_f).T)  # [in, tokens]
    tb = tokens // TB
    kb = KO_BF * P
    xbf = xf[:kb].astype(ml_dtypes.bfloat16)
    # (ko ki) (tb t) -> tb ki ko t
    x_bf = np.ascontiguousarray(
        xbf.reshape(KO_BF, P, tb, TB).transpose(2, 1, 0, 3)
    )
    x8f = xf[kb:].astype(ml_dtypes.float8_e4m3fn)
    # (d j ki) (tb t) -> tb ki d j t
    x_f8 = np.ascontiguousarray(
        x8f.reshape(D8, 2, P, tb, TB).transpose(3, 2, 0, 1, 4)
    )
    return x_bf, x_f8


def stage_pop_slice(pop_c: np.ndarray):
    """pop slice [out_c, in, 32] (+-1.0 f32) -> 2-bit-packed swarm planes
    [8 chunk, 128 p, ko, out_c] u8. Lossless bit-repack: plane q's byte
    holds swarm bits {q, 8+q, 16+q, 24+q} at bit positions 0/2/4/6, so
    summing <=3 planes keeps every 2-bit field carry-free."""
    out_c, in_f, _ = pop_c.shape
    ko = in_f // P
    bits = (pop_c > 0).astype(np.uint8).transpose(2, 1, 0)  # [32, in, out_c]
    planes = (bits[0:8] | (bits[8:16] << 2) | (bits[16:24] << 4)
              | (bits[24:32] << 6))  # [8 q, in, out_c]
    # [8 q, (ko p) in, oc] -> [8 c, 128 p, ko, oc]
    arr = planes.reshape(8, ko, P, out_c).transpose(0, 2, 1, 3)
    return np.ascontiguousarray(arr)


def unstage_y(y_dev: np.ndarray, tokens: int, out_c: int):
    """y^T [tb, 128 o, g, TB t] bf16 -> [tokens, out_c] f32
    (token = tb*TB + t, out = g*128 + o)."""
    return (
        y_dev.astype(np.float32)
        .transpose(0, 3, 2, 1)
        .reshape(tokens, out_c)
    )


def prep_inputs(x: np.ndarray, population: np.ndarray):
    tokens = x.shape[0] * x.shape[1]
    in_f = x.shape[2]
    x_bf, x_f8 = stage_x(x, tokens, in_f)
    out_c = population.shape[0] // N_CORES
    in_maps = []
    for c in range(N_CORES):
        pop_c = stage_pop_slice(population[c * out_c:(c + 1) * out_c])
        in_maps.append({"xT": x_bf, "x8": x_f8, "pop": pop_c})
    return in_maps, tokens, out_c, in_f


def kernel(x: np.ndarray, population: np.ndarray):
    in_maps, tokens, out_c, in_f = prep_inputs(x, population)
    nc = _get_nc(tokens, out_c, in_f)
    res = run_bass_kernel_spmd(nc, in_maps, core_ids=list(range(N_CORES)))
    y_full = np.concatenate(
        [unstage_y(r["y"], tokens, out_c) for r in res.results], axis=1
    )
    return y_full.reshape(x.shape[0], x.shape[1], population.shape[0])


# revision 24
# speedup vs baseline: 1.0489x; 1.0188x over previous
"""BitSwarmLinear Trainium2 kernel.

Computation (reference):
    swarm_sum = population.sum(axis=2)          # (out, in)
    w_eff     = sign(swarm_sum), sign(0) -> +1  # (out, in), +-1
    y         = einsum("bsi,oi->bso", x, w_eff) # (4, 4096, out)

Distribution (8 NeuronCores, tensor-parallel on out_features):
    - population sharded on out_features: each core gets its 256 rows,
      reduces + binarizes them and computes its 256 output columns.
    - x replicated to every core, staged pre-transposed/tiled so the
      contraction dim lands on SBUF partitions with fully-contiguous DMA.
    - outputs gathered on the host along the feature dim.

Precision/speed split (the key trick): the PE's only >1x datatype path on
TRN2 is fp8e4/e5 + perf_mode=DoubleRow (2 weights/cell, 2 MACs/cycle).
Full e4m3 x quantization costs 2.66% rel err (> the 2e-2 gate), so the
contraction is split: in-features [0,1024) stay bf16 (8 k-tiles), and
in-features [1024,2048) are e4m3 (4 DoubleRow double-tiles, rhs free dim
1024 = the fp8 moving max). Measured rel err 1.89e-2; PE cycles per
512-token PSUM group drop 8288 -> 6464 (0.78x) and x HBM traffic drops
64 MB -> 48 MB per core (the baseline ran at ~91% DMA occupancy, so both
rooflines move together).

Host staging (lossless / layout-only for pop; x is cast to bf16/e4m3):
    - population bits 2-bit-packed: plane q's byte holds swarm bits
      {q, 8+q, 16+q, 24+q} at bit positions 0/2/4/6, laid out IN-major
      [plane, in%128, in//128, out] so the reduction output lands directly
      in matmul-rhs orientation (no PE transpose). 4.2 MB/core (4x less
      than u8 planes).
    - x -> bf16 x^T tiles [tb, 128 ki, 8 ko, TB tok] for the bf16 half and
      e4m3 x^T pair-tiles [tb, 128 ki, 4 d, 2 j, TB tok] for the fp8 half
      (j indexes the DoubleRow k-pair slot; contiguous lines either way).
    - y returns bf16 tile-major; host restores [b, s, out] f32.

Per-core device pipeline:
    1. Eight pop plane DMAs (525 KB each, alternating SP/ACT HWDGE rings,
       all issued upfront). DVE sums plane sets A={p0..p2}, B={p3..p5},
       C={p6,p7} as uint16 lanes (2-bit fields stay <= 3: carry-free,
       exact in the fp32 ALU), then unpacks 2->4 bit (nibbles <= 12) and
       4->8 bit (bytes <= 32 = the swarm popcount). The whole chain is
       split by ko-half with the fp8 half first, so w_f8 closes ~10 us
       before w_bf.
    2. Binarize in ONE op per W half: w = (count >= 16) - 0.5 = +-0.5
       (sign(0)->+1 preserved; +-0.5 exact in e4m3/bf16); the PSUM-drain
       cast multiplies y by 2 (exact power-of-2, zero precision cost).
    3. ~30 garbage warm-up matmuls on dead pop bytes run from ~19 us (when
       the fp8-half B-set closes) so the HAM clock gate is at K=8/8 before
       any real matmul. The first two token blocks' DoubleRow matmuls are
       emitted ahead of everything (start=True, PSUM groups left open,
       g-major): they start as soon as w_f8 + the sliver-gated x8 tiles
       land and cover the bf16-half reduction. Body: per (g, blk) PSUM
       group, 4 DR MMs then 8 bf16 MMs (PE queue in-order; fp8 operands
       are ready first). DVE drains PSUM (x2 scale, bf16); y stores
       alternate HWDGE rings.
"""

import os
import sys

import numpy as np

for _p in ("/root/.axon_site/_ro/trn_rl_repo", "/opt/trn_rl_repo"):
    if os.path.isdir(_p) and _p not in sys.path:
        sys.path.append(_p)

import ml_dtypes

# bass_utils' axon trace path imports antenv.axon_hooks, which this image
# lacks. Provide it (backed by the ctypes NTFF hook) so running with
# BASS_TRACE=1 works instead of crashing on the import.
try:
    import antenv.axon_hooks  # noqa: F401
except ImportError:
    try:
        import types as _types

        from trn_agent_boot.trn_boot import _ntff_profile_via_ctypes

        _hooks = _types.ModuleType("antenv.axon_hooks")
        _ntff_hook = _ntff_profile_via_ctypes("/opt/axon/libaxon_pjrt.so")
        _hooks.get_axon_ntff_profile_hook = lambda: _ntff_hook
        _hooks.set_axon_ntff_profile_hook = lambda h: None
        sys.modules["antenv.axon_hooks"] = _hooks
    except Exception:
        pass

import concourse.bass as bass  # noqa: F401  (AP helpers)
import concourse.mybir as mybir
import concourse.tile as tile
from concourse import bacc
from concourse.bass_utils import run_bass_kernel_spmd

P = 128
IN_F = 2048
SWARM = 32
OUT_F = 2048
N_CORES = 8
OUT_C = OUT_F // N_CORES  # 256 out features per core
TOKENS = 4 * 4096

F32 = mybir.dt.float32
BF16 = mybir.dt.bfloat16
F8E4 = mybir.dt.float8e4
U8 = mybir.dt.uint8
U16 = mybir.dt.uint16

# token-block per x tile / output store
TB = 1024
# x prefetch depth
XT_BUFS = 4
# 2-bit-packed pop planes, one per chunk DMA
PK_CHUNKS = 8
# contraction split: k-tiles 0..KO_BF-1 bf16, the rest e4m3 DoubleRow pairs
KO_BF = 8
D8 = (16 - KO_BF) // 2  # fp8 double-tiles (2 k-tiles each)
DR = mybir.MatmulPerfMode.DoubleRow


def build_nc(tokens: int = TOKENS, out_c: int = OUT_C, in_f: int = IN_F,
             reps: int = 1):
    """Build the per-core Bass program (same program on all 8 cores).

    reps>1 repeats the whole pipeline back-to-back (timing harness only)."""
    ko_tiles = in_f // P          # 16 K-tiles
    tb_count = tokens // TB

    nc = bacc.Bacc(
        "TRN2",
        target_bir_lowering=False,
        debug=False,
        enable_asserts=False,
        num_devices=N_CORES,
    )

    xT = nc.dram_tensor("xT", [tb_count, P, KO_BF, TB], BF16,
                        kind="ExternalInput")
    x8 = nc.dram_tensor("x8", [tb_count, P, D8, 2, TB], F8E4,
                        kind="ExternalInput")
    pop = nc.dram_tensor("pop", [PK_CHUNKS, P, ko_tiles, out_c],
                         U8, kind="ExternalInput")
    # y^T tile-major: [tb, out%128, out//128, tok-in-block]
    y = nc.dram_tensor("y", [tb_count, P, out_c // P, TB], BF16,
                       kind="ExternalOutput")

    xr = xT.ap()                                              # [tb,128,8,TB]
    x8r = x8.ap()                                             # [tb,128,4,2,TB]
    pr = pop.ap()                                             # [8,128,2,ko,oc]
    yr = y.ap()                                               # [tb,128,m,TB]

    with tile.TileContext(nc) as tc:
        with (
            tc.tile_pool(name="pops", bufs=PK_CHUNKS) as pop_pool,
            tc.tile_pool(name="red", bufs=1) as red_pool,
            tc.tile_pool(name="wsb", bufs=1) as w_pool,
            tc.tile_pool(name="xt", bufs=XT_BUFS) as x_pool,
            tc.tile_pool(name="x8t", bufs=XT_BUFS) as x8_pool,
            tc.tile_pool(name="ystage", bufs=3) as y_pool,
            tc.tile_pool(name="psum_y", bufs=8, space="PSUM") as psum_pool,
        ):
            for _rep in range(reps):
                _emit_body(
                    nc, pop_pool, red_pool, w_pool, x_pool, x8_pool, y_pool,
                    psum_pool, pr, xr, x8r, yr, ko_tiles, tb_count,
                    out_c, in_f,
                )

    nc.compile()  # bacc register allocation / DCE — required before codegen
    return nc


def _emit_body(nc, pop_pool, red_pool, w_pool, x_pool, x8_pool, y_pool,
               psum_pool, pr, xr, x8r, yr, ko_tiles, tb_count, out_c, in_f):
    # W tiles: bf16 [in(part), ko, out] + e4m3 pairs [in(part), d, j, out]
    w_bf = w_pool.tile([P, KO_BF, out_c], BF16, tag="wbf")
    w_f8 = w_pool.tile([P, D8, 2, out_c], F8E4, tag="wf8")

    # ---- Stage 1: swarm reduction over 2-bit-packed planes (all DVE).
    # Plane q's byte holds swarm bits {q, 8+q, 16+q, 24+q} at bit positions
    # 0/2/4/6. Sets A={p0,p1,p2}, B={p3,p4,p5}, C={p6,p7} are summed as
    # uint16 lane views (2-bit fields stay <= 3 per set -> no carries;
    # everything is exact in the fp32 ALU). The whole chain is split by
    # ko-half with the fp8 half (k-tiles 8..15) FIRST, so w_f8 closes
    # ~10 us before w_bf and the early DoubleRow matmuls fill that window.
    oc_l = out_c // 2  # uint16 lanes per ko row
    accA = red_pool.tile([P, ko_tiles, oc_l], U16, tag="accA")
    accB = red_pool.tile([P, ko_tiles, oc_l], U16, tag="accB")
    accC = red_pool.tile([P, ko_tiles, oc_l], U16, tag="accC")
    uA = red_pool.tile([P, ko_tiles, oc_l], U16, tag="uA")
    uB = red_pool.tile([P, ko_tiles, oc_l], U16, tag="uB")
    uC = red_pool.tile([P, ko_tiles, oc_l], U16, tag="uC")
    q1 = red_pool.tile([P, ko_tiles, oc_l], U16, tag="q1")
    q2 = red_pool.tile([P, ko_tiles, oc_l], U16, tag="q2")
    tS = red_pool.tile([P, ko_tiles, oc_l], U16, tag="tS")

    SHR = mybir.AluOpType.logical_shift_right
    AND = mybir.AluOpType.bitwise_and
    ISGE = mybir.AluOpType.is_ge
    MUL = mybir.AluOpType.mult
    SUB = mybir.AluOpType.subtract

    # All 8 chunk DMAs issue upfront (bufs=8 -> no buffer-reuse
    # throttling). The x-tile head loads are sliver-gated on the reduction
    # chain: an ungated DMA's transfer round-robins with the pop chunks at
    # packet granularity across DMA queues and steals pop bandwidth.
    pk_tiles = []
    for c in range(PK_CHUNKS):
        pt = pop_pool.tile([P, ko_tiles, out_c], U8, tag="pops")
        eng = nc.sync if c % 2 == 0 else nc.scalar
        eng.dma_start(pt[:], pr[c])
        pk_tiles.append(pt)
    pv = [t[:].bitcast(U16) for t in pk_tiles]

    xt_head = [x_pool.tile([P, KO_BF, TB], BF16, tag="xt", name=f"xth{i}")
               for i in range(XT_BUFS)]
    x8_head = [x8_pool.tile([P, D8, 2, TB], F8E4, tag="x8t", name=f"x8h{i}")
               for i in range(XT_BUFS)]

    def load_x8(x8t, tb, gate=None):
        if gate is not None:
            nc.gpsimd.tensor_copy(out=x8t[0:1, 0, 0, 0:32].bitcast(U16),
                                  in_=gate)
        eng = nc.sync if tb % 2 == 0 else nc.scalar
        eng.dma_start(x8t[:], x8r[tb])

    def load_xt(xt, tb, gate=None):
        half = KO_BF // 2
        for h in range(2):
            if gate is not None:
                nc.gpsimd.tensor_copy(out=xt[0:1, h * 4, 0:16],
                                      in_=gate.bitcast(BF16))
            eng = nc.sync if h == 0 else nc.scalar
            eng.dma_start(xt[:, h * half:(h + 1) * half, :],
                          xr[tb, :, h * half:(h + 1) * half, :])

    hl = oc_l // 2  # 64 lanes per out-column half
    cnt_u8 = q1[:].bitcast(U8)  # [128, ko, out_c] counts in [0, 32]

    def unpack2(dst, src, ks):
        nc.vector.tensor_scalar(out=tS[:, ks], in0=src[:, ks], scalar1=2,
                                scalar2=0x3333, op0=SHR, op1=AND)
        nc.vector.tensor_scalar(out=dst[:, ks], in0=src[:, ks],
                                scalar1=0x3333, scalar2=None, op0=AND)
        nc.vector.tensor_add(dst[:, ks], dst[:, ks], tS[:, ks])

    def half_tail(ks, g):
        """abq (<=24) + cq (<=8) + cnt (<=32) for one (ko-half, out-half)."""
        ls = slice(g * hl, (g + 1) * hl)
        nc.vector.tensor_scalar(out=tS[:, ks, ls], in0=uB[:, ks, ls],
                                scalar1=4, scalar2=0x0F0F, op0=SHR, op1=AND)
        nc.vector.tensor_scalar(out=q1[:, ks, ls], in0=uB[:, ks, ls],
                                scalar1=0x0F0F, scalar2=None, op0=AND)
        nc.vector.tensor_add(q1[:, ks, ls], q1[:, ks, ls], tS[:, ks, ls])
        nc.vector.tensor_scalar(out=tS[:, ks, ls], in0=uC[:, ks, ls],
                                scalar1=4, scalar2=0x0F0F, op0=SHR, op1=AND)
        nc.vector.tensor_scalar(out=q2[:, ks, ls], in0=uC[:, ks, ls],
                                scalar1=0x0F0F, scalar2=None, op0=AND)
        nc.vector.tensor_add(q2[:, ks, ls], q2[:, ks, ls], tS[:, ks, ls])
        nc.vector.tensor_add(q1[:, ks, ls], q1[:, ks, ls], q2[:, ks, ls])

    def binarize(dst, src):
        # count >= 16 <=> swarm_sum >= 0. W is stored as +-0.5 (exact in
        # e4m3/bf16) in ONE op; the PSUM-drain cast multiplies y by 2
        # (exact power-of-2 scaling, no precision change).
        nc.vector.tensor_scalar(out=dst, in0=src, scalar1=16, scalar2=0.5,
                                op0=ISGE, op1=SUB)

    ksF = slice(KO_BF, ko_tiles)   # fp8 half: k-tiles 8..15
    ksB = slice(0, KO_BF)          # bf16 half
    # fp8-half chain first, with the bf16-half set adds slotted into the
    # chunk-arrival stalls of the fp8 chain; the unpack/tail for the bf16
    # half runs after the fp8 W has closed (the early DR matmuls cover it).
    nc.vector.tensor_add(accA[:, ksF], pv[0][:, ksF], pv[1][:, ksF])
    nc.vector.tensor_add(accA[:, ksB], pv[0][:, ksB], pv[1][:, ksB])
    nc.vector.tensor_add(accA[:, ksF], accA[:, ksF], pv[2][:, ksF])
    nc.vector.tensor_add(accA[:, ksB], accA[:, ksB], pv[2][:, ksB])
    unpack2(uA, accA, ksF)                              # A1 (F)
    nc.vector.tensor_add(accB[:, ksF], pv[3][:, ksF], pv[4][:, ksF])
    nc.vector.tensor_add(accB[:, ksF], accB[:, ksF], pv[5][:, ksF])
    nc.vector.tensor_add(accC[:, ksF], pv[6][:, ksF], pv[7][:, ksF])
    # fp8-half pop is fully consumed here: release the first x tiles
    gateF = accC[0:1, KO_BF, 0:16]
    load_x8(x8_head[0], 0, gateF)
    load_x8(x8_head[1], 1, gateF)
    load_xt(xt_head[0], 0, gateF)
    load_xt(xt_head[1], 1, gateF)
    unpack2(uB, accB, ksF)                              # B1 (F)
    nc.vector.tensor_add(uB[:, ksF], uB[:, ksF], uA[:, ksF])   # ab <= 12
    unpack2(uC, accC, ksF)                              # C1 (F) <= 4
    gateF2 = uC[0:1, KO_BF, 0:16]
    for i in range(2, XT_BUFS):
        load_x8(x8_head[i], i, gateF2)
        load_xt(xt_head[i], i, gateF2)
    for g in range(2):
        gs = slice(g * P, (g + 1) * P)
        half_tail(ksF, g)
        binarize(w_f8[:, :, :, gs], cnt_u8[:, KO_BF:ko_tiles, gs])
    # bf16-half set adds + unpack + tails (under early-DR matmul cover)
    nc.vector.tensor_add(accB[:, ksB], pv[3][:, ksB], pv[4][:, ksB])
    nc.vector.tensor_add(accB[:, ksB], accB[:, ksB], pv[5][:, ksB])
    nc.vector.tensor_add(accC[:, ksB], pv[6][:, ksB], pv[7][:, ksB])
    unpack2(uA, accA, ksB)
    unpack2(uB, accB, ksB)
    nc.vector.tensor_add(uB[:, ksB], uB[:, ksB], uA[:, ksB])
    unpack2(uC, accC, ksB)
    for g in range(2):
        gs = slice(g * P, (g + 1) * P)
        half_tail(ksB, g)
        binarize(w_bf[:, :, gs], cnt_u8[:, 0:KO_BF, gs])

    # ---- HAM warm-up: garbage matmuls on dead pop bytes keep the PE
    # busy from when the fp8-half B-set closes (~18 us) until the early
    # DR matmuls start, so HAM is at K=8/8 for the entire real body.
    # PSUM garbage is never read: the real group reusing the bank opens
    # with start=True. lhsT/rhs read pk0 / accB's fp8-half (both dead or
    # read-only afterwards), so nothing in the pipeline is delayed.
    wu_lhsT = pk_tiles[0][:, 0, 0:256].bitcast(BF16)
    wu_rhs = accB[:, KO_BF:KO_BF + 4, :].bitcast(BF16)
    # shares the first early-DR group's PSUM bank: that group's start=True
    # erases the garbage (has_written cleared), so no 9th bank is needed
    ps_d = psum_pool.tile([P, 512], F32, tag="yps")
    for _ in range(30):
        nc.tensor.matmul(ps_d[:], wu_lhsT, wu_rhs, start=True, stop=True)

    # ---- Stage 2.5: early DoubleRow matmuls. w_f8 + the x8 tiles land
    # while the bf16 W half + xt0 are still in flight, so the first
    # ED_TBS token blocks' DR matmuls are emitted ahead of everything:
    # they open the PSUM accumulation groups (start=True, no stop), fill
    # the PE's otherwise-idle head window, and warm HAM before the body.
    oc_g = out_c // P
    n_blk = TB // 512
    ED_TBS = 2  # 2 tb x 4 groups = all 8 PSUM banks held open
    ps_early = {}
    # g-major: the g0 DR matmuls run while w_f8's g1 half still finalizes
    for g in range(oc_g):
        for tb in range(ED_TBS):
            for blk in range(n_blk):
                if (g, tb, blk) == (0, 0, 0):
                    ps = ps_d  # reuse the warm-up bank
                else:
                    ps = psum_pool.tile([P, 512], F32, tag="yps")
                ps_early[(tb, g, blk)] = ps
                for dd in range(D8):
                    nc.tensor.matmul(
                        ps[:],
                        w_f8[:, dd, :, g * P:(g + 1) * P],
                        x8_head[tb][:, dd, :, blk * 512:(blk + 1) * 512],
                        start=(dd == 0),
                        stop=False,
                        perf_mode=DR,
                    )

    # ---- Stage 3: stream x tiles, matmul, store y (bf16)
    for tb in range(tb_count):
        if tb < XT_BUFS:
            xt = xt_head[tb]   # loads already issued in the head
            x8t = x8_head[tb]
        else:
            xt = x_pool.tile([P, KO_BF, TB], BF16, tag="xt")
            x8t = x8_pool.tile([P, D8, 2, TB], F8E4, tag="x8t")
            load_x8(x8t, tb)
            load_xt(xt, tb)
        # W-stationary matmuls: 4 e4m3 DoubleRow double-tiles + 8 bf16
        # k-tiles accumulate one PSUM bank [128 out, 512 tok]. DR first:
        # x8/w_f8 are ready before xt/w_bf, and the PE queue is in-order.
        ystage = y_pool.tile([P, oc_g, TB], BF16, tag="ys")
        for g in range(oc_g):
            for blk in range(n_blk):
                if tb < ED_TBS:
                    ps = ps_early[(tb, g, blk)]  # DR part already emitted
                else:
                    ps = psum_pool.tile([P, 512], F32, tag="yps")
                    for dd in range(D8):
                        nc.tensor.matmul(
                            ps[:],
                            w_f8[:, dd, :, g * P:(g + 1) * P],
                            x8t[:, dd, :, blk * 512:(blk + 1) * 512],
                            start=(dd == 0),
                            stop=False,
                            perf_mode=DR,
                        )
                for k in range(KO_BF):
                    nc.tensor.matmul(
                        ps[:],
                        w_bf[:, k, g * P:(g + 1) * P],
                        xt[:, k, blk * 512:(blk + 1) * 512],
                        start=False,
                        stop=(k == KO_BF - 1),
                    )
                nc.vector.tensor_scalar(
                    out=ystage[:, g, blk * 512:(blk + 1) * 512], in0=ps[:],
                    scalar1=2.0, scalar2=None, op0=mybir.AluOpType.mult,
                )
            eng = nc.sync if (tb + g) % 2 == 0 else nc.scalar
            eng.dma_start(yr[tb, :, g:g + 1, :], ystage[:, g:g + 1, :])


_NC_CACHE: dict = {}


def _get_nc(tokens=TOKENS, out_c=OUT_C, in_f=IN_F):
    key = (tokens, out_c, in_f)
    if key not in _NC_CACHE:
        _NC_CACHE[key] = build_nc(*key)
    return _NC_CACHE[key]


def stage_x(x: np.ndarray, tokens: int, in_f: int):
    """x [b, s, in] f32 -> (bf16 tiles [tb, 128 ki, 8 ko, TB] for
    in-features [0, 1024), e4m3 pair-tiles [tb, 128 ki, 4 d, 2 j, TB] for
    in-features [1024, 2048))."""
    xf = np.ascontiguousarray(x.reshape(tokens, in_f).T)  # [in, tokens]
    tb = tokens // TB
    kb = KO_BF * P
    xbf = xf[:kb].astype(ml_dtypes.bfloat16)
    # (ko ki) (tb t) -> tb ki ko t
    x_bf = np.ascontiguousarray(
        xbf.reshape(KO_BF, P, tb, TB).transpose(2, 1, 0, 3)
    )
    x8f = xf[kb:].astype(ml_dtypes.float8_e4m3fn)
    # (d j ki) (tb t) -> tb ki d j t
    x_f8 = np.ascontiguousarray(
        x8f.reshape(D8, 2, P, tb, TB).transpose(3, 2, 0, 1, 4)
    )
    return x_bf, x_f8


def stage_pop_slice(pop_c: np.ndarray):
    """pop slice [out_c, in, 32] (+-1.0 f32) -> 2-bit-packed swarm planes
    [8 chunk, 128 p, ko, out_c] u8. Lossless bit-repack: plane q's byte
    holds swarm bits {q, 8+q, 16+q, 24+q} at bit positions 0/2/4/6, so
    summing <=3 planes keeps every 2-bit field carry-free."""
    out_c, in_f, _ = pop_c.shape
    ko = in_f // P
    bits = (pop_c > 0).astype(np.uint8).transpose(2, 1, 0)  # [32, in, out_c]
    planes = (bits[0:8] | (bits[8:16] << 2) | (bits[16:24] << 4)
              | (bits[24:32] << 6))  # [8 q, in, out_c]
    # [8 q, (ko p) in, oc] -> [8 c, 128 p, ko, oc]
    arr = planes.reshape(8, ko, P, out_c).transpose(0, 2, 1, 3)
    return np.ascontiguousarray(arr)


def unstage_y(y_dev: np.ndarray, tokens: int, out_c: int):
    """y^T [tb, 128 o, g, TB t] bf16 -> [tokens, out_c] f32
    (token = tb*TB + t, out = g*128 + o)."""
    return (
        y_dev.astype(np.float32)
        .transpose(0, 3, 2, 1)
        .reshape(tokens, out_c)
    )


def prep_inputs(x: np.ndarray, population: np.ndarray):
    tokens = x.shape[0] * x.shape[1]
    in_f = x.shape[2]
    x_bf, x_f8 = stage_x(x, tokens, in_f)
    out_c = population.shape[0] // N_CORES
    in_maps = []
    for c in range(N_CORES):
        pop_c = stage_pop_slice(population[c * out_c:(c + 1) * out_c])
        in_maps.append({"xT": x_bf, "x8": x_f8, "pop": pop_c})
    return in_maps, tokens, out_c, in_f


def kernel(x: np.ndarray, population: np.ndarray):
    in_maps, tokens, out_c, in_f = prep_inputs(x, population)
    nc = _get_nc(tokens, out_c, in_f)
    res = run_bass_kernel_spmd(nc, in_maps, core_ids=list(range(N_CORES)))
    y_full = np.concatenate(
        [unstage_y(r["y"], tokens, out_c) for r in res.results], axis=1
    )
    return y_full.reshape(x.shape[0], x.shape[1], population.shape[0])
